# revision 10
# baseline (speedup 1.0000x reference)
"""DeepFactorRNN Trainium2 kernel.

Computes, for x = X.reshape(-1, F):
  mus    = sum_j(relu(LSTM2g(LSTM1g(x))) @ aff_W.T + aff_b)_j
  sigmas = softplus(relu(LSTM2n(LSTM1n(x))) @ noise_W.T + noise_b) + 1e-6
where each LSTM is a single step from zero state (so the forget gate is
unused and c = sigmoid(i)*tanh(g), h = sigmoid(o)*tanh(c)).

Strategy (8 NeuronCores, data parallel over the 32768 flattened rows):
 - Rows live on the matmul free dim; features/gates on partitions, so the
   whole network is transpose-free. X is transposed/cast on host.
 - f-gates are dropped from all weight matrices (25% matmul savings).
 - The aff linear + sum collapses to one dot with w_mu = aff_W.sum(0).
 - bf16 matmul operands, fp32 PSUM accumulation, fp32 activation math.
"""

import numpy as np
import ml_dtypes

BF16 = ml_dtypes.bfloat16

NCORES = 8
NTS, NPER, F = 128, 256, 128
GH, NH = 512, 256
ROWS = NTS * NPER            # 32768
RPC = ROWS // NCORES         # 4096 rows per core
RT = 1024                    # rows per tile
NT = RPC // RT               # 4 tiles per core
HALF = 512                   # matmul moving free-dim max

_CACHE = {}


def _build_program():
    import concourse.bacc as bacc
    import concourse.tile as tile
    from concourse import mybir

    dt = mybir.dt
    AFT = mybir.ActivationFunctionType

    nc = bacc.Bacc("TRN2", target_bir_lowering=False, debug=False,
                   num_devices=NCORES)

    # ---- DRAM I/O ----
    d_xT = nc.dram_tensor("xT", [F, RPC], dt.bfloat16, kind="ExternalInput")
    d_w0g = nc.dram_tensor("w0g", [F, 3 * GH], dt.bfloat16, kind="ExternalInput")
    d_w1g = nc.dram_tensor("w1g", [GH, 3 * GH], dt.bfloat16, kind="ExternalInput")
    d_w0n = nc.dram_tensor("w0n", [F, 3 * NH], dt.bfloat16, kind="ExternalInput")
    d_w1n = nc.dram_tensor("w1n", [NH, 3 * NH], dt.bfloat16, kind="ExternalInput")
    d_wmu = nc.dram_tensor("wmu", [128, GH // 128], dt.bfloat16, kind="ExternalInput")
    d_wsig = nc.dram_tensor("wsig", [128, NH // 128], dt.bfloat16, kind="ExternalInput")
    d_bg0 = nc.dram_tensor("bg0", [128, 3 * GH // 128], dt.float32, kind="ExternalInput")
    d_bg1 = nc.dram_tensor("bg1", [128, 3 * GH // 128], dt.float32, kind="ExternalInput")
    d_bn0 = nc.dram_tensor("bn0", [128, 3 * NH // 128], dt.float32, kind="ExternalInput")
    d_bn1 = nc.dram_tensor("bn1", [128, 3 * NH // 128], dt.float32, kind="ExternalInput")
    d_bsig = nc.dram_tensor("bsig", [1, 1], dt.float32, kind="ExternalInput")
    d_mus = nc.dram_tensor("mus_o", [1, RPC], dt.float32, kind="ExternalOutput")
    d_sig = nc.dram_tensor("sig_o", [1, RPC], dt.float32, kind="ExternalOutput")

    CG = GH // 128   # 4 chunks for global hidden
    CN = NH // 128   # 2 chunks for noise hidden

    with tile.TileContext(nc) as tc:
        with (
            tc.tile_pool(name="wp", bufs=1) as wp,
            tc.tile_pool(name="gp", bufs=2) as gp,
            tc.tile_pool(name="hp", bufs=2 * CG) as hp,
            tc.tile_pool(name="pp", bufs=4, space="PSUM") as pp,
        ):
            # ---- resident loads ----
            xT = wp.tile([F, RPC], dt.bfloat16, name="xT_sb")
            nc.sync.dma_start(out=xT, in_=d_xT[:, :])
            w0g = wp.tile([F, 3 * GH], dt.bfloat16, name="w0g_sb")
            nc.sync.dma_start(out=w0g, in_=d_w0g[:, :])
            w1g = [wp.tile([128, 3 * GH], dt.bfloat16, name=f"w1g_sb{k}")
                   for k in range(CG)]
            for k in range(CG):
                nc.sync.dma_start(out=w1g[k], in_=d_w1g[k * 128:(k + 1) * 128, :])
            w0n = wp.tile([F, 3 * NH], dt.bfloat16, name="w0n_sb")
            nc.sync.dma_start(out=w0n, in_=d_w0n[:, :])
            w1n = [wp.tile([128, 3 * NH], dt.bfloat16, name=f"w1n_sb{k}")
                   for k in range(CN)]
            for k in range(CN):
                nc.sync.dma_start(out=w1n[k], in_=d_w1n[k * 128:(k + 1) * 128, :])
            wmu = wp.tile([128, CG], dt.bfloat16, name="wmu_sb")
            nc.sync.dma_start(out=wmu, in_=d_wmu[:, :])
            wsig = wp.tile([128, CN], dt.bfloat16, name="wsig_sb")
            nc.sync.dma_start(out=wsig, in_=d_wsig[:, :])
            bg0 = wp.tile([128, 3 * CG], dt.float32, name="bg0_sb")
            nc.sync.dma_start(out=bg0, in_=d_bg0[:, :])
            bg1 = wp.tile([128, 3 * CG], dt.float32, name="bg1_sb")
            nc.sync.dma_start(out=bg1, in_=d_bg1[:, :])
            bn0 = wp.tile([128, 3 * CN], dt.float32, name="bn0_sb")
            nc.sync.dma_start(out=bn0, in_=d_bn0[:, :])
            bn1 = wp.tile([128, 3 * CN], dt.float32, name="bn1_sb")
            nc.sync.dma_start(out=bn1, in_=d_bn1[:, :])
            bsig = wp.tile([1, 1], dt.float32, name="bsig_sb")
            nc.sync.dma_start(out=bsig, in_=d_bsig[:, :])

            mu_full = wp.tile([1, RPC], dt.float32, name="mu_full")
            zs_full = wp.tile([1, RPC], dt.float32, name="zs_full")

            def gate_chunk(t, c, C, rhs_list, w_list, b_sb, out_tag, relu):
                """One 128-wide hidden chunk of an LSTM step for RT rows.

                rhs_list: K-dim chunks of the layer input, each [128, RT].
                Returns h = sigmoid(o) * tanh(c) (relu-folded if relu=True),
                as a [128, RT] bf16 tile.
                """
                nk = len(rhs_list)
                ps = []
                for gi in range(3):  # i, g, o
                    p = pp.tile([128, RT], dt.float32, tag="ps", bufs=4,
                                name=f"p_{out_tag}_{t}_{c}_{gi}")
                    mcol = (gi * C + c) * 128
                    for k in range(nk):
                        for h in range(RT // HALF):
                            hs = slice(h * HALF, (h + 1) * HALF)
                            nc.tensor.matmul(
                                p[:, hs],
                                w_list[k][:, mcol:mcol + 128],
                                rhs_list[k][:, hs],
                                start=(k == 0), stop=(k == nk - 1),
                            )
                    ps.append(p)
                pi, pg, po = ps
                ti = gp.tile([128, RT], dt.bfloat16, tag="ti", name=f"ti_{out_tag}_{t}_{c}")
                nc.scalar.activation(ti, pi, AFT.Sigmoid, bias=b_sb[:, c:c + 1])
                tg = gp.tile([128, RT], dt.bfloat16, tag="tg", name=f"tg_{out_tag}_{t}_{c}")
                nc.scalar.activation(tg, pg, AFT.Tanh, bias=b_sb[:, C + c:C + c + 1])
                to = gp.tile([128, RT], dt.bfloat16, tag="to", name=f"to_{out_tag}_{t}_{c}")
                nc.scalar.activation(to, po, AFT.Sigmoid, bias=b_sb[:, 2 * C + c:2 * C + c + 1])
                cc = gp.tile([128, RT], dt.bfloat16, tag="cc", name=f"cc_{out_tag}_{t}_{c}")
                nc.vector.tensor_mul(cc, ti, tg)
                if relu:
                    # relu(sig(o)*tanh(c)) == sig(o)*tanh(relu(c))
                    rc = gp.tile([128, RT], dt.bfloat16, tag="rc", name=f"rc_{out_tag}_{t}_{c}")
                    nc.vector.tensor_scalar_max(rc, cc, 0.0)
                    cc = rc
                th = gp.tile([128, RT], dt.bfloat16, tag="th", name=f"th_{out_tag}_{t}_{c}")
                nc.scalar.activation(th, cc, AFT.Tanh)
                hv = hp.tile([128, RT], dt.bfloat16, tag=out_tag, name=f"h_{out_tag}_{t}_{c}")
                nc.vector.tensor_mul(hv, to, th)
                return hv

            for t in range(NT):
                xt = xT[:, t * RT:(t + 1) * RT]

                # global branch
                h0g = [gate_chunk(t, c, CG, [xt], [w0g], bg0, "h0g", False)
                       for c in range(CG)]
                r1g = [gate_chunk(t, c, CG, h0g, w1g, bg1, "r1g", True)
                       for c in range(CG)]
                pmu = pp.tile([1, RT], dt.float32, tag="ps", bufs=4, name=f"pmu_{t}")
                for k in range(CG):
                    for h in range(RT // HALF):
                        hs = slice(h * HALF, (h + 1) * HALF)
                        nc.tensor.matmul(pmu[:, hs], wmu[:, k:k + 1],
                                         r1g[k][:, hs],
                                         start=(k == 0), stop=(k == CG - 1))
                nc.vector.tensor_scalar_add(
                    mu_full[:, t * RT:(t + 1) * RT], pmu, 0.0)

                # noise branch
                h0n = [gate_chunk(t, c, CN, [xt], [w0n], bn0, "h0n", False)
                       for c in range(CN)]
                r1n = [gate_chunk(t, c, CN, h0n, w1n, bn1, "r1n", True)
                       for c in range(CN)]
                psg = pp.tile([1, RT], dt.float32, tag="ps", bufs=4, name=f"psg_{t}")
                for k in range(CN):
                    for h in range(RT // HALF):
                        hs = slice(h * HALF, (h + 1) * HALF)
                        nc.tensor.matmul(psg[:, hs], wsig[:, k:k + 1],
                                         r1n[k][:, hs],
                                         start=(k == 0), stop=(k == CN - 1))
                nc.vector.tensor_scalar_add(
                    zs_full[:, t * RT:(t + 1) * RT], psg, 0.0)

            # tails: softplus(z+b) = ln(1+exp(z+b)); Exp and Ln share one
            # table set, and both run after every Sigmoid/Tanh in program
            # order, so the kernel pays a single table switch
            spe = wp.tile([1, RPC], dt.float32, name="spe_full")
            nc.scalar.activation(spe, zs_full, AFT.Exp, bias=bsig[:, 0:1])
            sp = wp.tile([1, RPC], dt.float32, name="sp_full")
            nc.scalar.activation(sp, spe, AFT.Ln, bias=1.0)
            nc.sync.dma_start(out=d_mus[:, :], in_=mu_full)
            nc.sync.dma_start(out=d_sig[:, :], in_=sp)

    nc.compile()
    return nc


def _pack_lstm_weights(W, b, H):
    """Drop the f gate; pack [i, g, o] along the output dim.
    Returns lhsT (K, 3H) bf16 and bias tile (128, 3H/128) f32."""
    idx = np.r_[0:H, 2 * H:3 * H, 3 * H:4 * H]
    Wp = W[idx]                      # (3H, K)
    bp = b[idx]                      # (3H,)
    lhsT = np.ascontiguousarray(Wp.T).astype(BF16)
    btile = np.ascontiguousarray(bp.reshape(3 * H // 128, 128).T).astype(np.float32)
    return lhsT, btile


def _make_in_maps(inputs):
    """Host-side packing: shard X, drop f-gates, fold aff into one dot.
    Returns (per-core input maps, summed aff bias to add on host)."""
    X = np.asarray(inputs["X"], np.float32)
    g_Wih0 = np.asarray(inputs["g_Wih0"], np.float32)
    g_b0 = np.asarray(inputs["g_b0"], np.float32)
    g_Wih1 = np.asarray(inputs["g_Wih1"], np.float32)
    g_b1 = np.asarray(inputs["g_b1"], np.float32)
    aff_W = np.asarray(inputs["aff_W"], np.float32)
    aff_b = np.asarray(inputs["aff_b"], np.float32)
    n_Wih0 = np.asarray(inputs["n_Wih0"], np.float32)
    n_b0 = np.asarray(inputs["n_b0"], np.float32)
    n_Wih1 = np.asarray(inputs["n_Wih1"], np.float32)
    n_b1 = np.asarray(inputs["n_b1"], np.float32)
    noise_W = np.asarray(inputs["noise_W"], np.float32)
    noise_b = np.asarray(inputs["noise_b"], np.float32)

    w0g, bg0 = _pack_lstm_weights(g_Wih0, g_b0, GH)
    w1g, bg1 = _pack_lstm_weights(g_Wih1, g_b1, GH)
    w0n, bn0 = _pack_lstm_weights(n_Wih0, n_b0, NH)
    w1n, bn1 = _pack_lstm_weights(n_Wih1, n_b1, NH)

    wm = aff_W.sum(axis=0)                     # (GH,)
    wmu = np.ascontiguousarray(wm.reshape(GH // 128, 128).T).astype(BF16)
    b_mu = float(aff_b.sum())
    ws = noise_W[0]                            # (NH,)
    wsig = np.ascontiguousarray(ws.reshape(NH // 128, 128).T).astype(BF16)
    bsig = np.array([[noise_b[0]]], np.float32)

    Xf = X.reshape(ROWS, F)
    shared = {
        "w0g": w0g, "w1g": w1g, "w0n": w0n, "w1n": w1n,
        "wmu": wmu, "wsig": wsig,
        "bg0": bg0, "bg1": bg1, "bn0": bn0, "bn1": bn1, "bsig": bsig,
    }
    in_maps = []
    for c in range(NCORES):
        xc = np.ascontiguousarray(
            Xf[c * RPC:(c + 1) * RPC].T).astype(BF16)    # (F, RPC)
        in_maps.append({"xT": xc, **shared})
    return in_maps, b_mu


def kernel(**inputs):
    from concourse.bass_utils import run_bass_kernel_spmd

    in_maps, b_mu = _make_in_maps(inputs)
    if "nc" not in _CACHE:
        _CACHE["nc"] = _build_program()
    nc = _CACHE["nc"]

    res = run_bass_kernel_spmd(nc, in_maps, list(range(NCORES)))

    mus = np.empty(ROWS, np.float32)
    sig = np.empty(ROWS, np.float32)
    for c in range(NCORES):
        mus[c * RPC:(c + 1) * RPC] = res.results[c]["mus_o"][0]
        sig[c * RPC:(c + 1) * RPC] = res.results[c]["sig_o"][0]
    # device returns mus without the (constant) summed bias, and sigmas'
    # softplus output without the +1e-6 epsilon; both folded here
    mus = (mus + b_mu).reshape(NTS, NPER)
    sig = (sig + 1e-6).reshape(NTS, NPER)
    return mus, sig
